# revision 42
# baseline (speedup 1.0000x reference)
"""Trainium2 Bass kernel for nn_MemoryAccess (scatter_memory).

Two SPMD launches with host glue between them (argmax + gather cannot be
fused on-device without cross-core collectives):

Launch 1 (8 cores): FA_r forward only as far as the device needs it for the
read-score matmul.  h = x@(32*w1) fp8; e = exp(h/32) f16 is EXPORTED and the
host rebuilds a = (e/s @ w2 + b2) * x in float64 with full-precision w2 — so
the device's second FA matmul can run in fp8 (it only has to rank top-32
candidates, worst observed true-argmax rank is 11).  z = ((e@32*(w2+1(x)b2))
* x) @ 64*W_read with W_read column-sharded 3750/core.  Bias folds
(w2' = w2 + ones(x)b2) remove every softmax-sum dependency from the device
critical path.  DMAs are consolidated (4 input tensors, wsh chunked) because
each DMA costs ~650ns sequencer issue + ~625ns shared-HWDGE slot + 900ns
completion-semaphore latency.

Host: softmax normalize, exact top-32 rescore + argmax, tanh; FA_u gate
branch entirely on host (it is 0.03% of module FLOPs); memory-row gather;
(1-u)*r precompute.

Launch 2 (1 core): FA_um batched over 48 rows + the 3-step FA_am recurrence.
The x-part of FA_um's first matmul is hoisted to the host
(xc = x@32*w1[256:] + 32*b1) and re-injected as a rank-16 one-hot
contraction, so only w1[:256] ships to the device.  All per-sample softmax
scales collapse to one reciprocal + one row-broadcast matmul per block, and
relu*scale fuses into a single scalar_tensor_tensor whose in1 reads the
broadcast straight from PSUM.  Weight DMAs are ordered in exact consumption
order (umw1r, umw2, wum, amw1, amw2, wam) so transfers pipeline with the
serial FA chain.

Quantization (validated vs reference in fp64 simulation, rel 6.07e-3):
fp8e3 x32 on all first-matmul weights, fp8e3 on umw2'/wum, bf16 on
amw2'/wam, f16 e export.  PSUM accumulation groups are emitted contiguously
per region (real TRN2 drops interleaved open groups in a bank).
"""

import numpy as np
import ml_dtypes

import concourse.bass as bass
import concourse.tile as tile
import concourse.mybir as mybir
from concourse import bacc, bass_utils

F32 = mybir.dt.float32
F16 = mybir.dt.float16
BF16 = mybir.dt.bfloat16
FP8 = mybir.dt.float8e3
AF = mybir.ActivationFunctionType
ALU = mybir.AluOpType

B, IN_CH, SLOTS, SLOT_SIZE, READ_SLOTS = 16, 512, 10000, 256, 3
N_CORES = 8
TOTAL_COLS = READ_SLOTS * SLOTS          # 30000
SHARD = TOTAL_COLS // N_CORES            # 3750
SHARD_PAD = 3840                         # 30 p-tiles of 128
CU, CA = IN_CH + SLOT_SIZE, 2 * SLOT_SIZE  # 768, 512
P3 = READ_SLOTS * B                      # 48

WS = 32.0                                # fp8 weight pre-scale (host)
WS_READ = 64.0                           # W_read pre-scale (host)

# wf8 flat-column layout (launch 2)
OFF_UMW1R = 0                            # [2 kt x 768]
OFF_UMW2 = 1536                          # [6 kt x 768]
OFF_WUM = 6144                           # [6 kt x 256]
OFF_AMW1 = 7680                          # [4 kt x 512]
WF8_COLS = 9728
# wbf flat-column layout
OFF_AMW2 = 0                             # [4 kt x 512]
OFF_WAM = 2048                           # [4 kt x 256]
WBF_COLS = 3072


def _prelude(nc, persist, bc_scale=1.0):
    onc_bf = persist.tile([128, 1], BF16, tag="onc_bf")
    nc.vector.memset(onc_bf[:, :], 1.0)
    # Broadcast-matmul lhsT; its value carries the 1/WS^2 descale so the
    # f16 row operands stay in normal range (4e-9 rows flush to zero).
    ones16 = persist.tile([1, 128], F16, tag="ones16")
    nc.vector.memset(ones16[:, :], bc_scale)
    # Touch Exp early so LoadActFuncSet overlaps the initial DMAs.
    t = persist.tile([1, 2], F32, tag="actwarm")
    nc.vector.memset(t[:, :], 0.0)
    nc.scalar.activation(t[:, :], t[:, :], AF.Exp)
    return onc_bf, ones16


def _build_launch1(with_b1):
    nc = bacc.Bacc("TRN2", target_bir_lowering=False, debug=False,
                   num_devices=N_CORES)
    d = {}
    for name, shape, dt in [
        ("xw", [128, 4, B], F16),          # x^T
        ("rw1", [128, 4, IN_CH], FP8),     # 32*fa_r_w1
        ("rw2", [128, 4, IN_CH], FP8),     # 32*(fa_r_w2 + 1(x)b2)
        ("wsh", [128, 4, SHARD_PAD], FP8),  # 64*W_read shard
    ]:
        d[name] = nc.dram_tensor(name, shape, dt, kind="ExternalInput").ap()
    if with_b1:
        d["bp"] = nc.dram_tensor("bp", [1, 4 * IN_CH], F32,
                                 kind="ExternalInput").ap()
    e_out = nc.dram_tensor("e", [128, 4, B], F16, kind="ExternalOutput").ap()
    z_out = nc.dram_tensor("z", [128, 30, B], BF16, kind="ExternalOutput").ap()

    CH = [(0, 15), (15, 27), (27, 30)]  # wsh p-tile chunks

    with tile.TileContext(nc) as tc:
        with (
            tc.tile_pool(name="persist", bufs=1) as persist,
            tc.tile_pool(name="psA", bufs=2, space="PSUM") as psA,
            tc.tile_pool(name="psZ", bufs=1, space="PSUM") as psZ,
            tc.tile_pool(name="psD", bufs=1, space="PSUM") as psD,
        ):
            onc_bf, ones16 = _prelude(nc, persist)
            zrow = persist.tile([1, 128], BF16, tag="zrow")
            nc.vector.memset(zrow[:, :], 0.0)

            # DMA order: rw1/xw up front (FA head), rw2 mid-stream between
            # wsh chunks (P only gates z, which rides the stream anyway),
            # small last wsh chunk so the z tail work is minimal.
            rw1 = persist.tile([128, 4, IN_CH], FP8, tag="rw1")
            nc.sync.dma_start(out=rw1[:, :, :], in_=d["rw1"][:, :, :])
            xw = persist.tile([128, 4, B], F16, tag="xw")
            nc.sync.dma_start(out=xw[:, :, :], in_=d["xw"][:, :, :])
            if with_b1:
                bp = persist.tile([1, 4 * IN_CH], F32, tag="bp")
                nc.sync.dma_start(out=bp[:, :], in_=d["bp"][:, :])
            wsh = persist.tile([128, 4, SHARD_PAD], FP8, tag="wsh")
            rw2 = persist.tile([128, 4, IN_CH], FP8, tag="rw2")
            (a0, a1) = CH[0]
            nc.sync.dma_start(out=wsh[:, :, a0 * 128:a1 * 128],
                              in_=d["wsh"][:, :, a0 * 128:a1 * 128])
            nc.sync.dma_start(out=rw2[:, :, :], in_=d["rw2"][:, :, :])
            for (t0, t1) in CH[1:]:
                nc.sync.dma_start(out=wsh[:, :, t0 * 128:t1 * 128],
                                  in_=d["wsh"][:, :, t0 * 128:t1 * 128])

            # leading dummy: absorb the first stall on a rhs wait (xw
            # arrives just after rw1, so the h Ldweights never block)
            dmy = psD.tile([1, B], F32, tag="dmy")
            nc.tensor.matmul(dmy[:, :], onc_bf[0:1, 0:1], xw[0:1, 0, :],
                             start=True, stop=True)

            # h = x @ 32*w1 (+ 32*b1); the zero-valued extra contraction row
            # (rhs = an rw2 slice) delays e until rw2 is resident so the P
            # Ldweights never block on the rw2 DMA.
            hp = psA.tile([128, 4, B], F32, tag="mm")
            for t in range(4):
                ops = [(rw1[:, k, t * 128:(t + 1) * 128], xw[:, k, :])
                       for k in range(4)]
                if with_b1:
                    ops.append((bp[:, t * 128:(t + 1) * 128], ones16[:, :B]))
                if t == 3:
                    ops.append((zrow[:, :], rw2[0:1, 3, IN_CH - B:IN_CH]))
                for j, (lh, rh) in enumerate(ops):
                    nc.tensor.matmul(hp[:, t, :], lh, rh, start=(j == 0),
                                     stop=(j == len(ops) - 1))
            e16 = persist.tile([128, 4, B], F16, tag="e16")
            nc.scalar.activation(e16[:, :, :], hp[:, :, :], AF.Exp,
                                 scale=1.0 / WS)
            nc.sync.dma_start(out=e_out[:, :, :], in_=e16[:, :, :])

            # P = e @ 32*(w2 + 1(x)b2)  (softmax sum folded out)
            pp = psA.tile([128, 4, B], F32, tag="mm")
            for t in range(4):
                for k in range(4):
                    nc.tensor.matmul(pp[:, t, :],
                                     rw2[:, k, t * 128:(t + 1) * 128],
                                     e16[:, k, :], start=(k == 0),
                                     stop=(k == 3))
            qb = persist.tile([128, 4, B], BF16, tag="qb")
            nc.vector.tensor_mul(qb[:, :, :], pp[:, :, :], xw[:, :, :])

            # z = qb @ wsh, chunk-pipelined with the wsh DMAs
            z_ps = psZ.tile([128, 30, B], F32, tag="z")
            z_sb = persist.tile([128, 30, B], BF16, tag="z_sb")
            for ci, (t0, t1) in enumerate(CH):
                for t in range(t0, t1):
                    for k in range(4):
                        nc.tensor.matmul(z_ps[:, t, :],
                                         wsh[:, k, t * 128:(t + 1) * 128],
                                         qb[:, k, :], start=(k == 0),
                                         stop=(k == 3))
                nc.vector.tensor_copy(z_sb[:, t0:t1, :], z_ps[:, t0:t1, :])
                nc.sync.dma_start(out=z_out[:, t0:t1, :], in_=z_sb[:, t0:t1, :])

    nc.compile()
    return nc


def _build_launch2(with_bum, with_amb1, with_bam, dbg=False):
    nc = bacc.Bacc("TRN2", target_bir_lowering=False, debug=False,
                   num_devices=1)
    d = {}
    for name, shape, dt in [
        ("glue", [128, 10, P3], BF16),   # cat3t(6) | ru(2) | gamu,rw rows
        ("xco", [16, 816], BF16),        # xc blocks [16,6*128] | onehot
        ("wf8", [128, WF8_COLS], FP8),   # umw1r | umw2' | wum | amw1
        ("wbf", [128, WBF_COLS], BF16),  # amw2' | wam
    ]:
        d[name] = nc.dram_tensor(name, shape, dt, kind="ExternalInput").ap()
    if with_bum or with_amb1 or with_bam:
        # [32^2*b_um (0:256) | 32*fa_am_b1 (256:768) | 32^2*b_am (768:1024)]
        d["bp2"] = nc.dram_tensor("bp2", [1, 1024], F32,
                                  kind="ExternalInput").ap()
    m_out = nc.dram_tensor("m", [128, 2, B], F32, kind="ExternalOutput").ap()
    if dbg:
        dbg_outs = {
            "d_eum": nc.dram_tensor("d_eum", [128, 6, P3], BF16,
                                    kind="ExternalOutput").ap(),
            "d_r2b": nc.dram_tensor("d_r2b", [128, 2, P3], BF16,
                                    kind="ExternalOutput").ap(),
            "d_gam": nc.dram_tensor("d_gam", [1, P3], F16,
                                    kind="ExternalOutput").ap(),
            "d_bcg": nc.dram_tensor("d_bcg", [128, 2, P3], F32,
                                    kind="ExternalOutput").ap(),
            "d_e0": nc.dram_tensor("d_e0", [128, 4, B], BF16,
                                   kind="ExternalOutput").ap(),
            "d_fT0": nc.dram_tensor("d_fT0", [128, 4, B], BF16,
                                    kind="ExternalOutput").ap(),
            "d_bc0": nc.dram_tensor("d_bc0", [128, 2, B], F32,
                                    kind="ExternalOutput").ap(),
            "d_m0": nc.dram_tensor("d_m0", [128, 2, B], BF16,
                                   kind="ExternalOutput").ap(),
        }

    with tile.TileContext(nc) as tc:
        with (
            tc.tile_pool(name="persist", bufs=1) as persist,
            tc.tile_pool(name="psA", bufs=1, space="PSUM") as psA,
            tc.tile_pool(name="psS", bufs=1, space="PSUM") as psS,
            tc.tile_pool(name="psW", bufs=1, space="PSUM") as psW,
            tc.tile_pool(name="psB", bufs=1, space="PSUM") as psB,
        ):
            # pool aliases: 8 distinct PSUM tile tags == 8 banks total.
            psM = psM2 = psT = psY = psW
            psR = psB
            onc_bf, ones16 = _prelude(nc, persist, bc_scale=1.0 / (WS * WS))
            if with_amb1:
                ones_b = persist.tile([1, B], F16, tag="ones_b")
                nc.vector.memset(ones_b[:, :], 1.0)

            # ---- DMA order: each matmul group's weights land before its
            # rhs becomes available (blocking Ldweights reset the PE ramp).
            wf8 = persist.tile([128, WF8_COLS], FP8, tag="wf8")
            wbf = persist.tile([128, WBF_COLS], BF16, tag="wbf")
            glue = persist.tile([128, 10, P3], BF16, tag="glue")
            xco = persist.tile([16, 816], BF16, tag="xco")
            UMW2_MID = OFF_UMW2 + 3 * CU  # split umw2 for finer landing
            nc.sync.dma_start(out=wf8[:, OFF_UMW1R:OFF_UMW2],
                              in_=d["wf8"][:, OFF_UMW1R:OFF_UMW2])
            nc.sync.dma_start(out=xco[:, :], in_=d["xco"][:, :])
            if with_bum or with_amb1 or with_bam:
                bp2 = persist.tile([1, 1024], F32, tag="bp2")
                nc.sync.dma_start(out=bp2[:, :], in_=d["bp2"][:, :])
            nc.sync.dma_start(out=glue[:, :, :], in_=d["glue"][:, :, :])
            for c0, c1 in [(OFF_UMW2, UMW2_MID), (UMW2_MID, OFF_WUM),
                           (OFF_WUM, OFF_AMW1), (OFF_AMW1, WF8_COLS)]:
                nc.sync.dma_start(out=wf8[:, c0:c1], in_=d["wf8"][:, c0:c1])
            for c0, c1 in [(OFF_AMW2, OFF_WAM), (OFF_WAM, WBF_COLS)]:
                nc.sync.dma_start(out=wbf[:, c0:c1], in_=d["wbf"][:, c0:c1])

            # leading dummy mm, rhs-gated on xco (lands after umw1r)
            dmy2 = psS.tile([1, P3], F32, tag="s")
            nc.tensor.matmul(dmy2[:, 0:B], onc_bf[0:1, 0:1], xco[0:1, 0:B],
                             start=True, stop=True)

            cat3t = glue[:, 0:6, :]
            ru = glue[:, 6:8, :]
            gamu_row = glue[0:1, 8, :]
            rw_row = glue[0:1, 9, :]
            oh = xco[:, 768:816]

            # ---- FA_um over 48 rows ----
            # h: r-part (2 kt fp8) + host xc via rank-16 one-hot contraction
            hp = psA.tile([128, 6, P3], F32, tag="big")
            for t in range(6):
                for k in range(2):
                    nc.tensor.matmul(
                        hp[:, t, :],
                        wf8[:, OFF_UMW1R + k * CU + t * 128:
                            OFF_UMW1R + k * CU + (t + 1) * 128],
                        cat3t[:, k, :], start=(k == 0), stop=False)
                nc.tensor.matmul(hp[:, t, :],
                                 xco[:, t * 128:(t + 1) * 128],
                                 oh[:, :], start=False, stop=True)
            e_um = persist.tile([128, 6, P3], BF16, tag="e_um")
            nc.scalar.activation(e_um[:, :, :], hp[:, :, :], AF.Exp,
                                 scale=1.0 / WS)
            s_ps = psS.tile([1, P3], F32, tag="s")
            for k in range(6):
                nc.tensor.matmul(s_ps[:, :], onc_bf[:, :], e_um[:, k, :],
                                 start=(k == 0), stop=(k == 5))
            # P = e @ 32*(w2 + 1(x)b2)
            pp = psA.tile([128, 6, P3], F32, tag="big")
            for t in range(6):
                for k in range(6):
                    nc.tensor.matmul(
                        pp[:, t, :],
                        wf8[:, OFF_UMW2 + k * CU + t * 128:
                            OFF_UMW2 + k * CU + (t + 1) * 128],
                        e_um[:, k, :], start=(k == 0), stop=(k == 5))
            f_um = persist.tile([128, 6, P3], BF16, tag="f_um")
            nc.vector.tensor_mul(f_um[:, :, :], pp[:, :, :], cat3t[:, :, :])
            # T'' = F @ 32*W_um (+ s*1024*b_um via brow if nonzero)
            ss16 = None
            if with_bum or with_bam:
                ss16 = persist.tile([1, P3], F16, tag="ss16")
                nc.vector.tensor_copy(ss16[:, :], s_ps[:, :])
            tp = psM.tile([128, 2, P3], F32, tag="mm")
            for t in range(2):
                for k in range(6):
                    nc.tensor.matmul(
                        tp[:, t, :],
                        wf8[:, OFF_WUM + k * SLOT_SIZE + t * 128:
                            OFF_WUM + k * SLOT_SIZE + (t + 1) * 128],
                        f_um[:, k, :], start=(k == 0),
                        stop=(not with_bum and k == 5))
                if with_bum:
                    nc.tensor.matmul(tp[:, t, :],
                                     bp2[:, t * 128:(t + 1) * 128],
                                     ss16[:, :], start=False, stop=True)
            # gam = (u/1024) / s, broadcast via ones-mm; r2 = max(T,0)*gam + ru
            rs_um = persist.tile([1, P3], F32, tag="rs_um")
            nc.vector.reciprocal(rs_um[:, :], s_ps[:, :])
            gam16 = persist.tile([1, P3], F16, tag="gam16")
            nc.vector.tensor_mul(gam16[:, :], rs_um[:, :], gamu_row)
            bcg = psB.tile([128, 2, P3], F32, tag="b")
            for t in range(2):
                nc.tensor.matmul(bcg[:, t, :], ones16[:, :], gam16[:, :],
                                 start=True, stop=True)
            bcg_sb = persist.tile([128, 2, P3], F32, tag="bcg_sb")
            nc.vector.tensor_copy(bcg_sb[:, :, :], bcg[:, :, :])
            r2a = persist.tile([128, 2, P3], F32, tag="r2a")
            nc.vector.scalar_tensor_tensor(r2a[:, :, :], tp[:, :, :], 0.0,
                                           bcg_sb[:, :, :], ALU.max, ALU.mult)
            r2b = persist.tile([128, 2, P3], BF16, tag="r2b")
            nc.vector.tensor_add(r2b[:, :, :], r2a[:, :, :], ru)
            if dbg:
                nc.sync.dma_start(out=dbg_outs["d_eum"][:, :, :],
                                  in_=e_um[:, :, :])
                nc.sync.dma_start(out=dbg_outs["d_gam"][:, :],
                                  in_=gam16[:, :])
                nc.sync.dma_start(out=dbg_outs["d_bcg"][:, :, :],
                                  in_=bcg_sb[:, :, :])
                nc.sync.dma_start(out=dbg_outs["d_r2b"][:, :, :],
                                  in_=r2b[:, :, :])

            # ---- 3-step FA_am recurrence ----
            m_prev = None
            for i in range(READ_SLOTS):
                r2s = r2b[:, :, i * B:(i + 1) * B]
                kt_f = 4 if i > 0 else 2
                yp = psY.tile([128, 4, B], F32, tag="y")
                for t in range(4):
                    # assemble the contraction list for this output tile
                    ops = [(wf8[:, OFF_AMW1 + k * CA + t * 128:
                                OFF_AMW1 + k * CA + (t + 1) * 128],
                            r2s[:, k, :]) for k in range(2)]
                    if i > 0:
                        ops += [(wf8[:, OFF_AMW1 + (2 + k) * CA + t * 128:
                                    OFF_AMW1 + (2 + k) * CA + (t + 1) * 128],
                                 m_prev[:, k, :]) for k in range(2)]
                    if with_amb1:
                        ops.append((bp2[:, 256 + t * 128:256 + (t + 1) * 128],
                                    ones_b[:, :]))
                    for j, (lh, rh) in enumerate(ops):
                        nc.tensor.matmul(yp[:, t, :], lh, rh,
                                         start=(j == 0),
                                         stop=(j == len(ops) - 1))
                e = persist.tile([128, 4, B], BF16, tag=f"e{i}")
                nc.scalar.activation(e[:, :, :], yp[:, :, :], AF.Exp,
                                     scale=1.0 / WS)
                sp_t = psS.tile([1, P3], F32, tag="s")
                sp = sp_t[:, 0:B]
                for k in range(4):
                    nc.tensor.matmul(sp[:, :], onc_bf[:, :], e[:, k, :],
                                     start=(k == 0), stop=(k == 3))
                pp2 = psM2.tile([128, 4, B], F32, tag="mm2")
                for t in range(4):
                    for k in range(4):
                        nc.tensor.matmul(
                            pp2[:, t, :],
                            wbf[:, OFF_AMW2 + k * CA + t * 128:
                                OFF_AMW2 + k * CA + (t + 1) * 128],
                            e[:, k, :], start=(k == 0), stop=(k == 3))
                fT = persist.tile([128, 4, B], BF16, tag=f"fT{i}")
                nc.vector.tensor_mul(fT[:, 0:2, :], pp2[:, 0:2, :], r2s)
                if i > 0:
                    nc.vector.tensor_mul(fT[:, 2:4, :], pp2[:, 2:4, :],
                                         m_prev[:, :, :])
                ss2 = None
                if with_bam:
                    ss2 = persist.tile([1, B], F16, tag=f"ssb{i}")
                    nc.vector.tensor_copy(ss2[:, :], sp[:, :])
                tp2 = psT.tile([128, 2, B], F32, tag="mmt")
                for t in range(2):
                    for k in range(kt_f):
                        nc.tensor.matmul(
                            tp2[:, t, :],
                            wbf[:, OFF_WAM + k * SLOT_SIZE + t * 128:
                                OFF_WAM + k * SLOT_SIZE + (t + 1) * 128],
                            fT[:, k, :], start=(k == 0),
                            stop=(not with_bam and k == kt_f - 1))
                    if with_bam:
                        nc.tensor.matmul(
                            tp2[:, t, :],
                            bp2[:, 768 + t * 128:768 + (t + 1) * 128],
                            ss2[:, :], start=False, stop=True)
                rs3 = persist.tile([1, B], F32, tag=f"rs3{i}")
                nc.vector.reciprocal(rs3[:, :], sp[:, :])
                rwrs = persist.tile([1, B], F16, tag=f"rwrs{i}")
                nc.vector.tensor_mul(rwrs[:, :], rs3[:, :],
                                     rw_row[:, i * B:(i + 1) * B])
                bc2 = psR.tile([128, 2, B], F32, tag="r")
                for t in range(2):
                    nc.tensor.matmul(bc2[:, t, :], ones16[:, :], rwrs[:, :],
                                     start=True, stop=True)
                bc2_sb = persist.tile([128, 2, B], F32, tag=f"bc2sb{i}")
                nc.vector.tensor_copy(bc2_sb[:, :, :], bc2[:, :, :])
                if i == READ_SLOTS - 1:
                    m32 = persist.tile([128, 2, B], F32, tag="m32")
                    nc.vector.scalar_tensor_tensor(
                        m32[:, :, :], tp2[:, :, :], 0.0, bc2_sb[:, :, :],
                        ALU.max, ALU.mult)
                    nc.sync.dma_start(out=m_out[:, :, :], in_=m32[:, :, :])
                else:
                    m_prev = persist.tile([128, 2, B], BF16, tag=f"m{i}")
                    nc.vector.scalar_tensor_tensor(
                        m_prev[:, :, :], tp2[:, :, :], 0.0, bc2_sb[:, :, :],
                        ALU.max, ALU.mult)
                    if dbg and i == 0:
                        nc.sync.dma_start(out=dbg_outs["d_e0"][:, :, :],
                                          in_=e[:, :, :])
                        nc.sync.dma_start(out=dbg_outs["d_fT0"][:, :, :],
                                          in_=fT[:, :, :])
                        nc.sync.dma_start(out=dbg_outs["d_bc0"][:, :, :],
                                          in_=bc2_sb[:, :, :])
                        nc.sync.dma_start(out=dbg_outs["d_m0"][:, :, :],
                                          in_=m_prev[:, :, :])

    nc.compile()
    return nc


_CACHE = {}


def _get_kernels(flags=(False, False, False, False)):
    if flags not in _CACHE:
        _CACHE[flags] = (_build_launch1(flags[0]),
                         _build_launch2(flags[1], flags[2], flags[3]))
    return _CACHE[flags]


def _pack128(w, kt):
    """[K, N] -> [128, kt, N] (partition-major, k-tiles along free dim)."""
    K, N = w.shape
    assert K == kt * 128
    return np.ascontiguousarray(w.reshape(kt, 128, N).transpose(1, 0, 2))


def _packflat(w, kt):
    """[K, N] -> [128, kt*N]."""
    K, N = w.shape
    assert K == kt * 128
    return np.ascontiguousarray(
        w.reshape(kt, 128, N).transpose(1, 0, 2).reshape(128, kt * N))


def _softmax(h):
    e = np.exp(h - h.max(axis=1, keepdims=True))
    return e / e.sum(axis=1, keepdims=True)


def kernel(**inputs):
    f16 = np.float16
    bf = ml_dtypes.bfloat16
    f8 = ml_dtypes.float8_e3m4
    inp = {k: np.asarray(v, dtype=np.float32) for k, v in inputs.items()}
    x = inp["inputs"]                     # [16, 512]
    xT = np.ascontiguousarray(x.T)        # [512, 16]

    with_b1 = bool(np.abs(inp["fa_r_b1"]).max() > 0)
    with_bum = bool(np.abs(inp["b_um"]).max() > 0)
    with_amb1 = bool(np.abs(inp["fa_am_b1"]).max() > 0)
    with_bam = bool(np.abs(inp["b_am"]).max() > 0)
    nc1, nc2 = _get_kernels((with_b1, with_bum, with_amb1, with_bam))

    # ---- launch 1 ----
    rw2p = inp["fa_r_w2"] + np.outer(np.ones(IN_CH, np.float32),
                                     inp["fa_r_b2"])
    common = {
        "xw": _pack128(xT.astype(f16), 4),
        "rw1": _pack128((WS * inp["fa_r_w1"]).astype(f8), 4),
        "rw2": _pack128((WS * rw2p).astype(f8), 4),
    }
    if with_b1:
        common["bp"] = (WS * inp["fa_r_b1"]).reshape(1, -1)
    wpad = np.zeros((IN_CH, N_CORES * SHARD_PAD), dtype=f8)
    wsc = (WS_READ * inp["W_read"]).astype(f8)
    for c in range(N_CORES):
        wpad[:, c * SHARD_PAD:c * SHARD_PAD + SHARD] = \
            wsc[:, c * SHARD:(c + 1) * SHARD]
    in_maps = []
    for c in range(N_CORES):
        m = dict(common)
        m["wsh"] = _pack128(wpad[:, c * SHARD_PAD:(c + 1) * SHARD_PAD], 4)
        in_maps.append(m)
    res1 = bass_utils.run_bass_kernel_spmd(nc1, in_maps,
                                           core_ids=list(range(N_CORES)))

    # ---- host glue ----
    e16 = np.asarray(res1.results[0]["e"], np.float64)         # [128, 4, 16]
    eT = e16.transpose(1, 0, 2).reshape(IN_CH, B)              # [512, 16]
    s = eT.sum(axis=0)                                         # [16]
    a = ((eT / s).T @ inp["fa_r_w2"].astype(np.float64)
         + inp["fa_r_b2"]) * x.astype(np.float64)              # [16, 512]

    zs = []
    for c in range(N_CORES):
        zc = np.asarray(res1.results[c]["z"], np.float32)
        zs.append(zc.transpose(1, 0, 2).reshape(SHARD_PAD, B)[:SHARD])
    zraw = np.concatenate(zs, axis=0).astype(np.float64)       # [30000, 16]
    z = (zraw / (WS * WS_READ * s)).T + inp["b_read"]          # [16, 30000]
    zd = z.reshape(B, READ_SLOTS, SLOTS)

    k = 32
    cand = np.argpartition(-zd, k, axis=2)[:, :, :k]
    wr = inp["W_read"].astype(np.float64).reshape(IN_CH, READ_SLOTS, SLOTS)
    br = inp["b_read"].astype(np.float64).reshape(READ_SLOTS, SLOTS)
    idx = np.empty((B, READ_SLOTS), np.int64)
    val = np.empty((B, READ_SLOTS))
    for b_ in range(B):
        for i in range(READ_SLOTS):
            c = cand[b_, i]
            exact = a[b_] @ wr[:, i, c] + br[i, c]
            j = int(np.argmax(exact))
            idx[b_, i] = c[j]
            val[b_, i] = exact[j]
    read_w = np.tanh(val)                                      # [16, 3]

    # FA_u gate branch fully on host (f64)
    xd = x.astype(np.float64)
    hu = _softmax(xd @ inp["fa_u_w1"].astype(np.float64) + inp["fa_u_b1"])
    fu = xd * (hu @ inp["fa_u_w2"].astype(np.float64) + inp["fa_u_b2"])
    u = 1.0 / (1.0 + np.exp(-(fu @ inp["W_uw"].astype(np.float64)
                              + inp["b_uw"])))                 # [16, 3]

    r_all = inp["memory"][np.arange(B)[:, None], idx]          # [16, 3, 256]
    r_sm = r_all.transpose(1, 0, 2).reshape(P3, SLOT_SIZE)     # step-major
    u_sm = u.T.reshape(-1).astype(np.float32)                  # [48]
    rw_sm = read_w.T.reshape(-1).astype(np.float32)            # [48]

    # ---- launch 2 inputs ----
    cat3 = np.concatenate([r_sm, np.tile(x, (READ_SLOTS, 1))], axis=1)
    cat3T = np.ascontiguousarray(cat3.T)                       # [768, 48]
    ru = (1.0 - u_sm)[:, None] * r_sm                          # [48, 256]
    glue = np.zeros((128, 10, P3), dtype=bf)
    glue[:, 0:6, :] = _pack128(cat3T.astype(bf), 6)
    glue[:, 6:8, :] = _pack128(np.ascontiguousarray(ru.T).astype(bf), 2)
    # natural scale; the 1/WS^2 descale lives in the broadcast lhsT value
    glue[0, 8, :] = u_sm.astype(bf)
    glue[0, 9, :] = rw_sm.astype(bf)

    umw1 = WS * inp["fa_um_w1"]                                # [768, 768]
    xc = (x @ umw1[SLOT_SIZE:] + WS * inp["fa_um_b1"])         # [16, 768]
    xco = np.zeros((16, 816), dtype=bf)
    xco[:, 0:768] = xc.astype(bf)
    xco[:, 768:816] = np.tile(np.eye(B, dtype=bf), (1, READ_SLOTS))

    umw2p = WS * (inp["fa_um_w2"]
                  + np.outer(np.ones(CU, np.float32), inp["fa_um_b2"]))
    amw2p = WS * (inp["fa_am_w2"]
                  + np.outer(np.ones(CA, np.float32), inp["fa_am_b2"]))
    wf8 = np.concatenate([
        _packflat((umw1[:SLOT_SIZE]).astype(f8), 2),
        _packflat(umw2p.astype(f8), 6),
        _packflat((WS * inp["W_um"]).astype(f8), 6),
        _packflat((WS * inp["fa_am_w1"]).astype(f8), 4),
    ], axis=1)
    wbf = np.concatenate([
        _packflat(amw2p.astype(bf), 4),
        _packflat((WS * inp["W_am"]).astype(bf), 4),
    ], axis=1)
    in_map2 = {"glue": glue, "xco": xco, "wf8": wf8, "wbf": wbf}
    if with_bum or with_amb1 or with_bam:
        in_map2["bp2"] = np.concatenate([
            WS * WS * inp["b_um"], WS * inp["fa_am_b1"],
            WS * WS * inp["b_am"],
        ]).reshape(1, -1).astype(np.float32)
    res2 = bass_utils.run_bass_kernel_spmd(nc2, [in_map2], core_ids=[0])
    mT = np.asarray(res2.results[0]["m"], np.float32)          # [128, 2, 16]
    m = mT.transpose(1, 0, 2).reshape(SLOT_SIZE, B).T          # [16, 256]
    return np.tanh(m)


# revision 43
# speedup vs baseline: 1.0106x; 1.0106x over previous
"""Trainium2 Bass kernel for nn_MemoryAccess (scatter_memory).

Two SPMD launches with host glue between them (argmax + gather cannot be
fused on-device without cross-core collectives):

Launch 1 (8 cores): FA_r forward only as far as the device needs it for the
read-score matmul.  h = x@(32*w1) fp8; e = exp(h/32) f16 is EXPORTED and the
host rebuilds a = (e/s @ w2 + b2) * x in float64 with full-precision w2 — so
the device's second FA matmul can run in fp8 (it only has to rank top-32
candidates, worst observed true-argmax rank is 11).  z = ((e@32*(w2+1(x)b2))
* x) @ 64*W_read with W_read column-sharded 3750/core.  Bias folds
(w2' = w2 + ones(x)b2) remove every softmax-sum dependency from the device
critical path.  DMAs are consolidated (4 input tensors, wsh chunked) because
each DMA costs ~650ns sequencer issue + ~625ns shared-HWDGE slot + 900ns
completion-semaphore latency.

Host: softmax normalize, exact top-32 rescore + argmax, tanh; FA_u gate
branch entirely on host (it is 0.03% of module FLOPs); memory-row gather;
(1-u)*r precompute.

Launch 2 (1 core): FA_um batched over 48 rows + the 3-step FA_am recurrence.
The x-part of FA_um's first matmul is hoisted to the host
(xc = x@32*w1[256:] + 32*b1) and re-injected as a rank-16 one-hot
contraction, so only w1[:256] ships to the device.  All per-sample softmax
scales collapse to one reciprocal + one row-broadcast matmul per block, and
relu*scale fuses into a single scalar_tensor_tensor whose in1 reads the
broadcast straight from PSUM.  Weight DMAs are ordered in exact consumption
order (umw1r, umw2, wum, amw1, amw2, wam) so transfers pipeline with the
serial FA chain.

Quantization (validated vs reference in fp64 simulation, rel 6.07e-3):
fp8e3 x32 on all first-matmul weights, fp8e3 on umw2'/wum, bf16 on
amw2'/wam, f16 e export.  PSUM accumulation groups are emitted contiguously
per region (real TRN2 drops interleaved open groups in a bank).
"""

import numpy as np
import ml_dtypes

import concourse.bass as bass
import concourse.tile as tile
import concourse.mybir as mybir
from concourse import bacc, bass_utils

F32 = mybir.dt.float32
F16 = mybir.dt.float16
BF16 = mybir.dt.bfloat16
FP8 = mybir.dt.float8e3
AF = mybir.ActivationFunctionType
ALU = mybir.AluOpType

B, IN_CH, SLOTS, SLOT_SIZE, READ_SLOTS = 16, 512, 10000, 256, 3
N_CORES = 8
TOTAL_COLS = READ_SLOTS * SLOTS          # 30000
SHARD = TOTAL_COLS // N_CORES            # 3750
SHARD_PAD = 3840                         # 30 p-tiles of 128
CU, CA = IN_CH + SLOT_SIZE, 2 * SLOT_SIZE  # 768, 512
P3 = READ_SLOTS * B                      # 48

WS = 32.0                                # fp8 weight pre-scale (host)
WS_READ = 64.0                           # W_read pre-scale (host)

# wf8 flat-column layout (launch 2)
OFF_UMW1R = 0                            # [2 kt x 768]
OFF_UMW2 = 1536                          # [6 kt x 768]
OFF_WUM = 6144                           # [6 kt x 256]
OFF_AMW1 = 7680                          # [4 kt x 512]
WF8_COLS = 9728
# wbf flat-column layout
OFF_AMW2 = 0                             # [4 kt x 512]
OFF_WAM = 2048                           # [4 kt x 256]
WBF_COLS = 3072


def _prelude(nc, persist, bc_scale=1.0):
    onc_bf = persist.tile([128, 1], BF16, tag="onc_bf")
    nc.vector.memset(onc_bf[:, :], 1.0)
    # Broadcast-matmul lhsT; its value carries the 1/WS^2 descale so the
    # f16 row operands stay in normal range (4e-9 rows flush to zero).
    ones16 = persist.tile([1, 128], F16, tag="ones16")
    nc.vector.memset(ones16[:, :], bc_scale)
    # Touch Exp early so LoadActFuncSet overlaps the initial DMAs.
    t = persist.tile([1, 2], F32, tag="actwarm")
    nc.vector.memset(t[:, :], 0.0)
    nc.scalar.activation(t[:, :], t[:, :], AF.Exp)
    return onc_bf, ones16


def _build_launch1(with_b1):
    nc = bacc.Bacc("TRN2", target_bir_lowering=False, debug=False,
                   num_devices=N_CORES)
    d = {}
    for name, shape, dt in [
        ("xw", [128, 4, B], F16),          # x^T
        ("rw1", [128, 4, IN_CH], FP8),     # 32*fa_r_w1
        ("rw2", [128, 4, IN_CH], FP8),     # 32*(fa_r_w2 + 1(x)b2)
        ("wsh", [128, 4, SHARD_PAD], FP8),  # 64*W_read shard
    ]:
        d[name] = nc.dram_tensor(name, shape, dt, kind="ExternalInput").ap()
    if with_b1:
        d["bp"] = nc.dram_tensor("bp", [1, 4 * IN_CH], F32,
                                 kind="ExternalInput").ap()
    e_out = nc.dram_tensor("e", [128, 4, B], F16, kind="ExternalOutput").ap()
    z_out = nc.dram_tensor("z", [128, 30, B], BF16, kind="ExternalOutput").ap()

    CH = [(0, 15), (15, 27), (27, 30)]  # wsh p-tile chunks

    with tile.TileContext(nc) as tc:
        with (
            tc.tile_pool(name="persist", bufs=1) as persist,
            tc.tile_pool(name="psA", bufs=2, space="PSUM") as psA,
            tc.tile_pool(name="psZ", bufs=1, space="PSUM") as psZ,
            tc.tile_pool(name="psD", bufs=1, space="PSUM") as psD,
        ):
            onc_bf, ones16 = _prelude(nc, persist)
            zrow = persist.tile([1, 128], BF16, tag="zrow")
            nc.vector.memset(zrow[:, :], 0.0)

            # DMA order: rw1/xw up front (FA head), rw2 mid-stream between
            # wsh chunks (P only gates z, which rides the stream anyway),
            # small last wsh chunk so the z tail work is minimal.
            rw1 = persist.tile([128, 4, IN_CH], FP8, tag="rw1")
            nc.sync.dma_start(out=rw1[:, :, :], in_=d["rw1"][:, :, :])
            xw = persist.tile([128, 4, B], F16, tag="xw")
            nc.sync.dma_start(out=xw[:, :, :], in_=d["xw"][:, :, :])
            if with_b1:
                bp = persist.tile([1, 4 * IN_CH], F32, tag="bp")
                nc.sync.dma_start(out=bp[:, :], in_=d["bp"][:, :])
            wsh = persist.tile([128, 4, SHARD_PAD], FP8, tag="wsh")
            rw2 = persist.tile([128, 4, IN_CH], FP8, tag="rw2")
            (a0, a1) = CH[0]
            nc.sync.dma_start(out=wsh[:, :, a0 * 128:a1 * 128],
                              in_=d["wsh"][:, :, a0 * 128:a1 * 128])
            nc.sync.dma_start(out=rw2[:, :, :], in_=d["rw2"][:, :, :])
            for (t0, t1) in CH[1:]:
                nc.sync.dma_start(out=wsh[:, :, t0 * 128:t1 * 128],
                                  in_=d["wsh"][:, :, t0 * 128:t1 * 128])

            # leading dummy: absorb the first stall on a rhs wait (xw
            # arrives just after rw1, so the h Ldweights never block)
            dmy = psD.tile([1, B], F32, tag="dmy")
            nc.tensor.matmul(dmy[:, :], onc_bf[0:1, 0:1], xw[0:1, 0, :],
                             start=True, stop=True)

            # h = x @ 32*w1 (+ 32*b1); the zero-valued extra contraction row
            # (rhs = an rw2 slice) delays e until rw2 is resident so the P
            # Ldweights never block on the rw2 DMA.
            hp = psA.tile([128, 4, B], F32, tag="mm")
            for t in range(4):
                ops = [(rw1[:, k, t * 128:(t + 1) * 128], xw[:, k, :])
                       for k in range(4)]
                if with_b1:
                    ops.append((bp[:, t * 128:(t + 1) * 128], ones16[:, :B]))
                if t == 3:
                    ops.append((zrow[:, :], rw2[0:1, 3, IN_CH - B:IN_CH]))
                for j, (lh, rh) in enumerate(ops):
                    nc.tensor.matmul(hp[:, t, :], lh, rh, start=(j == 0),
                                     stop=(j == len(ops) - 1))
            e16 = persist.tile([128, 4, B], F16, tag="e16")
            nc.scalar.activation(e16[:, :, :], hp[:, :, :], AF.Exp,
                                 scale=1.0 / WS)
            nc.sync.dma_start(out=e_out[:, :, :], in_=e16[:, :, :])

            # P = e @ 32*(w2 + 1(x)b2)  (softmax sum folded out)
            pp = psA.tile([128, 4, B], F32, tag="mm")
            for t in range(4):
                for k in range(4):
                    nc.tensor.matmul(pp[:, t, :],
                                     rw2[:, k, t * 128:(t + 1) * 128],
                                     e16[:, k, :], start=(k == 0),
                                     stop=(k == 3))
            qb = persist.tile([128, 4, B], BF16, tag="qb")
            nc.vector.tensor_mul(qb[:, :, :], pp[:, :, :], xw[:, :, :])

            # z = qb @ wsh, chunk-pipelined with the wsh DMAs.  Separate PSUM
            # and SBUF tiles per chunk — a shared tile serializes the chunk
            # copies behind later chunks' matmuls (whole-tile dep tracking).
            for ci, (t0, t1) in enumerate(CH):
                nt = t1 - t0
                z_ps = psZ.tile([128, 15, B], F32, tag=f"z{ci}")
                z_sb = persist.tile([128, nt, B], BF16, tag=f"z_sb{ci}")
                for t in range(t0, t1):
                    for k in range(4):
                        nc.tensor.matmul(z_ps[:, t - t0, :],
                                         wsh[:, k, t * 128:(t + 1) * 128],
                                         qb[:, k, :], start=(k == 0),
                                         stop=(k == 3))
                nc.vector.tensor_copy(z_sb[:, :, :], z_ps[:, 0:nt, :])
                nc.sync.dma_start(out=z_out[:, t0:t1, :], in_=z_sb[:, :, :])

    nc.compile()
    return nc


def _build_launch2(with_bum, with_amb1, with_bam, dbg=False):
    nc = bacc.Bacc("TRN2", target_bir_lowering=False, debug=False,
                   num_devices=1)
    d = {}
    for name, shape, dt in [
        ("glue", [128, 10, P3], BF16),   # cat3t(6) | ru(2) | gamu,rw rows
        ("xco", [16, 816], BF16),        # xc blocks [16,6*128] | onehot
        ("wf8", [128, WF8_COLS], FP8),   # umw1r | umw2' | wum | amw1
        ("wbf", [128, WBF_COLS], BF16),  # amw2' | wam
    ]:
        d[name] = nc.dram_tensor(name, shape, dt, kind="ExternalInput").ap()
    if with_bum or with_amb1 or with_bam:
        # [32^2*b_um (0:256) | 32*fa_am_b1 (256:768) | 32^2*b_am (768:1024)]
        d["bp2"] = nc.dram_tensor("bp2", [1, 1024], F32,
                                  kind="ExternalInput").ap()
    m_out = nc.dram_tensor("m", [128, 2, B], F32, kind="ExternalOutput").ap()
    if dbg:
        dbg_outs = {
            "d_eum": nc.dram_tensor("d_eum", [128, 6, P3], BF16,
                                    kind="ExternalOutput").ap(),
            "d_r2b": nc.dram_tensor("d_r2b", [128, 2, P3], BF16,
                                    kind="ExternalOutput").ap(),
            "d_gam": nc.dram_tensor("d_gam", [1, P3], F16,
                                    kind="ExternalOutput").ap(),
            "d_bcg": nc.dram_tensor("d_bcg", [128, 2, P3], F32,
                                    kind="ExternalOutput").ap(),
            "d_e0": nc.dram_tensor("d_e0", [128, 4, B], BF16,
                                   kind="ExternalOutput").ap(),
            "d_fT0": nc.dram_tensor("d_fT0", [128, 4, B], BF16,
                                    kind="ExternalOutput").ap(),
            "d_bc0": nc.dram_tensor("d_bc0", [128, 2, B], F32,
                                    kind="ExternalOutput").ap(),
            "d_m0": nc.dram_tensor("d_m0", [128, 2, B], BF16,
                                   kind="ExternalOutput").ap(),
        }

    with tile.TileContext(nc) as tc:
        with (
            tc.tile_pool(name="persist", bufs=1) as persist,
            tc.tile_pool(name="psA", bufs=1, space="PSUM") as psA,
            tc.tile_pool(name="psS", bufs=1, space="PSUM") as psS,
            tc.tile_pool(name="psW", bufs=1, space="PSUM") as psW,
            tc.tile_pool(name="psB", bufs=1, space="PSUM") as psB,
        ):
            # pool aliases: 8 distinct PSUM tile tags == 8 banks total.
            psM = psM2 = psT = psY = psW
            psR = psB
            onc_bf, ones16 = _prelude(nc, persist, bc_scale=1.0 / (WS * WS))
            if with_amb1:
                ones_b = persist.tile([1, B], F16, tag="ones_b")
                nc.vector.memset(ones_b[:, :], 1.0)

            # ---- DMA order: each matmul group's weights land before its
            # rhs becomes available (blocking Ldweights reset the PE ramp).
            wf8 = persist.tile([128, WF8_COLS], FP8, tag="wf8")
            wbf = persist.tile([128, WBF_COLS], BF16, tag="wbf")
            glue = persist.tile([128, 10, P3], BF16, tag="glue")
            xco = persist.tile([16, 816], BF16, tag="xco")
            UMW2_MID = OFF_UMW2 + 3 * CU  # split umw2 for finer landing
            nc.sync.dma_start(out=wf8[:, OFF_UMW1R:OFF_UMW2],
                              in_=d["wf8"][:, OFF_UMW1R:OFF_UMW2])
            nc.sync.dma_start(out=xco[:, :], in_=d["xco"][:, :])
            if with_bum or with_amb1 or with_bam:
                bp2 = persist.tile([1, 1024], F32, tag="bp2")
                nc.sync.dma_start(out=bp2[:, :], in_=d["bp2"][:, :])
            nc.sync.dma_start(out=glue[:, :, :], in_=d["glue"][:, :, :])
            for c0, c1 in [(OFF_UMW2, UMW2_MID), (UMW2_MID, OFF_WUM),
                           (OFF_WUM, OFF_AMW1), (OFF_AMW1, WF8_COLS)]:
                nc.sync.dma_start(out=wf8[:, c0:c1], in_=d["wf8"][:, c0:c1])
            for c0, c1 in [(OFF_AMW2, OFF_WAM), (OFF_WAM, WBF_COLS)]:
                nc.sync.dma_start(out=wbf[:, c0:c1], in_=d["wbf"][:, c0:c1])

            # leading dummy mm, rhs-gated on xco (lands after umw1r)
            dmy2 = psS.tile([1, P3], F32, tag="s")
            nc.tensor.matmul(dmy2[:, 0:B], onc_bf[0:1, 0:1], xco[0:1, 0:B],
                             start=True, stop=True)

            cat3t = glue[:, 0:6, :]
            ru = glue[:, 6:8, :]
            gamu_row = glue[0:1, 8, :]
            rw_row = glue[0:1, 9, :]
            oh = xco[:, 768:816]

            # ---- FA_um over 48 rows ----
            # h: r-part (2 kt fp8) + host xc via rank-16 one-hot contraction
            hp = psA.tile([128, 6, P3], F32, tag="big")
            for t in range(6):
                for k in range(2):
                    nc.tensor.matmul(
                        hp[:, t, :],
                        wf8[:, OFF_UMW1R + k * CU + t * 128:
                            OFF_UMW1R + k * CU + (t + 1) * 128],
                        cat3t[:, k, :], start=(k == 0), stop=False)
                nc.tensor.matmul(hp[:, t, :],
                                 xco[:, t * 128:(t + 1) * 128],
                                 oh[:, :], start=False, stop=True)
            e_um = persist.tile([128, 6, P3], BF16, tag="e_um")
            nc.scalar.activation(e_um[:, :, :], hp[:, :, :], AF.Exp,
                                 scale=1.0 / WS)
            s_ps = psS.tile([1, P3], F32, tag="s")
            for k in range(6):
                nc.tensor.matmul(s_ps[:, :], onc_bf[:, :], e_um[:, k, :],
                                 start=(k == 0), stop=(k == 5))
            # P = e @ 32*(w2 + 1(x)b2)
            pp = psA.tile([128, 6, P3], F32, tag="big")
            for t in range(6):
                for k in range(6):
                    nc.tensor.matmul(
                        pp[:, t, :],
                        wf8[:, OFF_UMW2 + k * CU + t * 128:
                            OFF_UMW2 + k * CU + (t + 1) * 128],
                        e_um[:, k, :], start=(k == 0), stop=(k == 5))
            f_um = persist.tile([128, 6, P3], BF16, tag="f_um")
            nc.vector.tensor_mul(f_um[:, :, :], pp[:, :, :], cat3t[:, :, :])
            # T'' = F @ 32*W_um (+ s*1024*b_um via brow if nonzero)
            ss16 = None
            if with_bum or with_bam:
                ss16 = persist.tile([1, P3], F16, tag="ss16")
                nc.vector.tensor_copy(ss16[:, :], s_ps[:, :])
            tp = psM.tile([128, 2, P3], F32, tag="mm")
            for t in range(2):
                for k in range(6):
                    nc.tensor.matmul(
                        tp[:, t, :],
                        wf8[:, OFF_WUM + k * SLOT_SIZE + t * 128:
                            OFF_WUM + k * SLOT_SIZE + (t + 1) * 128],
                        f_um[:, k, :], start=(k == 0),
                        stop=(not with_bum and k == 5))
                if with_bum:
                    nc.tensor.matmul(tp[:, t, :],
                                     bp2[:, t * 128:(t + 1) * 128],
                                     ss16[:, :], start=False, stop=True)
            # gam = (u/1024) / s, broadcast via ones-mm; r2 = max(T,0)*gam + ru
            rs_um = persist.tile([1, P3], F32, tag="rs_um")
            nc.vector.reciprocal(rs_um[:, :], s_ps[:, :])
            gam16 = persist.tile([1, P3], F16, tag="gam16")
            nc.vector.tensor_mul(gam16[:, :], rs_um[:, :], gamu_row)
            bcg = psB.tile([128, 2, P3], F32, tag="b")
            for t in range(2):
                nc.tensor.matmul(bcg[:, t, :], ones16[:, :], gam16[:, :],
                                 start=True, stop=True)
            bcg_sb = persist.tile([128, 2, P3], F32, tag="bcg_sb")
            nc.vector.tensor_copy(bcg_sb[:, :, :], bcg[:, :, :])
            r2a = persist.tile([128, 2, P3], F32, tag="r2a")
            nc.vector.scalar_tensor_tensor(r2a[:, :, :], tp[:, :, :], 0.0,
                                           bcg_sb[:, :, :], ALU.max, ALU.mult)
            r2b = persist.tile([128, 2, P3], BF16, tag="r2b")
            nc.vector.tensor_add(r2b[:, :, :], r2a[:, :, :], ru)
            if dbg:
                nc.sync.dma_start(out=dbg_outs["d_eum"][:, :, :],
                                  in_=e_um[:, :, :])
                nc.sync.dma_start(out=dbg_outs["d_gam"][:, :],
                                  in_=gam16[:, :])
                nc.sync.dma_start(out=dbg_outs["d_bcg"][:, :, :],
                                  in_=bcg_sb[:, :, :])
                nc.sync.dma_start(out=dbg_outs["d_r2b"][:, :, :],
                                  in_=r2b[:, :, :])

            # ---- 3-step FA_am recurrence ----
            m_prev = None
            for i in range(READ_SLOTS):
                r2s = r2b[:, :, i * B:(i + 1) * B]
                kt_f = 4 if i > 0 else 2
                yp = psY.tile([128, 4, B], F32, tag="y")
                for t in range(4):
                    # assemble the contraction list for this output tile
                    ops = [(wf8[:, OFF_AMW1 + k * CA + t * 128:
                                OFF_AMW1 + k * CA + (t + 1) * 128],
                            r2s[:, k, :]) for k in range(2)]
                    if i > 0:
                        ops += [(wf8[:, OFF_AMW1 + (2 + k) * CA + t * 128:
                                    OFF_AMW1 + (2 + k) * CA + (t + 1) * 128],
                                 m_prev[:, k, :]) for k in range(2)]
                    if with_amb1:
                        ops.append((bp2[:, 256 + t * 128:256 + (t + 1) * 128],
                                    ones_b[:, :]))
                    for j, (lh, rh) in enumerate(ops):
                        nc.tensor.matmul(yp[:, t, :], lh, rh,
                                         start=(j == 0),
                                         stop=(j == len(ops) - 1))
                e = persist.tile([128, 4, B], BF16, tag=f"e{i}")
                nc.scalar.activation(e[:, :, :], yp[:, :, :], AF.Exp,
                                     scale=1.0 / WS)
                sp_t = psS.tile([1, P3], F32, tag="s")
                sp = sp_t[:, 0:B]
                for k in range(4):
                    nc.tensor.matmul(sp[:, :], onc_bf[:, :], e[:, k, :],
                                     start=(k == 0), stop=(k == 3))
                pp2 = psM2.tile([128, 4, B], F32, tag="mm2")
                for t in range(4):
                    for k in range(4):
                        nc.tensor.matmul(
                            pp2[:, t, :],
                            wbf[:, OFF_AMW2 + k * CA + t * 128:
                                OFF_AMW2 + k * CA + (t + 1) * 128],
                            e[:, k, :], start=(k == 0), stop=(k == 3))
                fT = persist.tile([128, 4, B], BF16, tag=f"fT{i}")
                nc.vector.tensor_mul(fT[:, 0:2, :], pp2[:, 0:2, :], r2s)
                if i > 0:
                    nc.vector.tensor_mul(fT[:, 2:4, :], pp2[:, 2:4, :],
                                         m_prev[:, :, :])
                ss2 = None
                if with_bam:
                    ss2 = persist.tile([1, B], F16, tag=f"ssb{i}")
                    nc.vector.tensor_copy(ss2[:, :], sp[:, :])
                tp2 = psT.tile([128, 2, B], F32, tag="mmt")
                for t in range(2):
                    for k in range(kt_f):
                        nc.tensor.matmul(
                            tp2[:, t, :],
                            wbf[:, OFF_WAM + k * SLOT_SIZE + t * 128:
                                OFF_WAM + k * SLOT_SIZE + (t + 1) * 128],
                            fT[:, k, :], start=(k == 0),
                            stop=(not with_bam and k == kt_f - 1))
                    if with_bam:
                        nc.tensor.matmul(
                            tp2[:, t, :],
                            bp2[:, 768 + t * 128:768 + (t + 1) * 128],
                            ss2[:, :], start=False, stop=True)
                rs3 = persist.tile([1, B], F32, tag=f"rs3{i}")
                nc.vector.reciprocal(rs3[:, :], sp[:, :])
                rwrs = persist.tile([1, B], F16, tag=f"rwrs{i}")
                nc.vector.tensor_mul(rwrs[:, :], rs3[:, :],
                                     rw_row[:, i * B:(i + 1) * B])
                bc2 = psR.tile([128, 2, B], F32, tag="r")
                for t in range(2):
                    nc.tensor.matmul(bc2[:, t, :], ones16[:, :], rwrs[:, :],
                                     start=True, stop=True)
                bc2_sb = persist.tile([128, 2, B], F32, tag=f"bc2sb{i}")
                nc.vector.tensor_copy(bc2_sb[:, :, :], bc2[:, :, :])
                if i == READ_SLOTS - 1:
                    m32 = persist.tile([128, 2, B], F32, tag="m32")
                    nc.vector.scalar_tensor_tensor(
                        m32[:, :, :], tp2[:, :, :], 0.0, bc2_sb[:, :, :],
                        ALU.max, ALU.mult)
                    nc.sync.dma_start(out=m_out[:, :, :], in_=m32[:, :, :])
                else:
                    m_prev = persist.tile([128, 2, B], BF16, tag=f"m{i}")
                    nc.vector.scalar_tensor_tensor(
                        m_prev[:, :, :], tp2[:, :, :], 0.0, bc2_sb[:, :, :],
                        ALU.max, ALU.mult)
                    if dbg and i == 0:
                        nc.sync.dma_start(out=dbg_outs["d_e0"][:, :, :],
                                          in_=e[:, :, :])
                        nc.sync.dma_start(out=dbg_outs["d_fT0"][:, :, :],
                                          in_=fT[:, :, :])
                        nc.sync.dma_start(out=dbg_outs["d_bc0"][:, :, :],
                                          in_=bc2_sb[:, :, :])
                        nc.sync.dma_start(out=dbg_outs["d_m0"][:, :, :],
                                          in_=m_prev[:, :, :])

    nc.compile()
    return nc


_CACHE = {}


def _get_kernels(flags=(False, False, False, False)):
    if flags not in _CACHE:
        _CACHE[flags] = (_build_launch1(flags[0]),
                         _build_launch2(flags[1], flags[2], flags[3]))
    return _CACHE[flags]


def _pack128(w, kt):
    """[K, N] -> [128, kt, N] (partition-major, k-tiles along free dim)."""
    K, N = w.shape
    assert K == kt * 128
    return np.ascontiguousarray(w.reshape(kt, 128, N).transpose(1, 0, 2))


def _packflat(w, kt):
    """[K, N] -> [128, kt*N]."""
    K, N = w.shape
    assert K == kt * 128
    return np.ascontiguousarray(
        w.reshape(kt, 128, N).transpose(1, 0, 2).reshape(128, kt * N))


def _softmax(h):
    e = np.exp(h - h.max(axis=1, keepdims=True))
    return e / e.sum(axis=1, keepdims=True)


def kernel(**inputs):
    f16 = np.float16
    bf = ml_dtypes.bfloat16
    f8 = ml_dtypes.float8_e3m4
    inp = {k: np.asarray(v, dtype=np.float32) for k, v in inputs.items()}
    x = inp["inputs"]                     # [16, 512]
    xT = np.ascontiguousarray(x.T)        # [512, 16]

    with_b1 = bool(np.abs(inp["fa_r_b1"]).max() > 0)
    with_bum = bool(np.abs(inp["b_um"]).max() > 0)
    with_amb1 = bool(np.abs(inp["fa_am_b1"]).max() > 0)
    with_bam = bool(np.abs(inp["b_am"]).max() > 0)
    nc1, nc2 = _get_kernels((with_b1, with_bum, with_amb1, with_bam))

    # ---- launch 1 ----
    rw2p = inp["fa_r_w2"] + np.outer(np.ones(IN_CH, np.float32),
                                     inp["fa_r_b2"])
    common = {
        "xw": _pack128(xT.astype(f16), 4),
        "rw1": _pack128((WS * inp["fa_r_w1"]).astype(f8), 4),
        "rw2": _pack128((WS * rw2p).astype(f8), 4),
    }
    if with_b1:
        common["bp"] = (WS * inp["fa_r_b1"]).reshape(1, -1)
    wpad = np.zeros((IN_CH, N_CORES * SHARD_PAD), dtype=f8)
    wsc = (WS_READ * inp["W_read"]).astype(f8)
    for c in range(N_CORES):
        wpad[:, c * SHARD_PAD:c * SHARD_PAD + SHARD] = \
            wsc[:, c * SHARD:(c + 1) * SHARD]
    in_maps = []
    for c in range(N_CORES):
        m = dict(common)
        m["wsh"] = _pack128(wpad[:, c * SHARD_PAD:(c + 1) * SHARD_PAD], 4)
        in_maps.append(m)
    res1 = bass_utils.run_bass_kernel_spmd(nc1, in_maps,
                                           core_ids=list(range(N_CORES)))

    # ---- host glue ----
    e16 = np.asarray(res1.results[0]["e"], np.float64)         # [128, 4, 16]
    eT = e16.transpose(1, 0, 2).reshape(IN_CH, B)              # [512, 16]
    s = eT.sum(axis=0)                                         # [16]
    a = ((eT / s).T @ inp["fa_r_w2"].astype(np.float64)
         + inp["fa_r_b2"]) * x.astype(np.float64)              # [16, 512]

    zs = []
    for c in range(N_CORES):
        zc = np.asarray(res1.results[c]["z"], np.float32)
        zs.append(zc.transpose(1, 0, 2).reshape(SHARD_PAD, B)[:SHARD])
    zraw = np.concatenate(zs, axis=0).astype(np.float64)       # [30000, 16]
    z = (zraw / (WS * WS_READ * s)).T + inp["b_read"]          # [16, 30000]
    zd = z.reshape(B, READ_SLOTS, SLOTS)

    k = 32
    cand = np.argpartition(-zd, k, axis=2)[:, :, :k]
    wr = inp["W_read"].astype(np.float64).reshape(IN_CH, READ_SLOTS, SLOTS)
    br = inp["b_read"].astype(np.float64).reshape(READ_SLOTS, SLOTS)
    idx = np.empty((B, READ_SLOTS), np.int64)
    val = np.empty((B, READ_SLOTS))
    for b_ in range(B):
        for i in range(READ_SLOTS):
            c = cand[b_, i]
            exact = a[b_] @ wr[:, i, c] + br[i, c]
            j = int(np.argmax(exact))
            idx[b_, i] = c[j]
            val[b_, i] = exact[j]
    read_w = np.tanh(val)                                      # [16, 3]

    # FA_u gate branch fully on host (f64)
    xd = x.astype(np.float64)
    hu = _softmax(xd @ inp["fa_u_w1"].astype(np.float64) + inp["fa_u_b1"])
    fu = xd * (hu @ inp["fa_u_w2"].astype(np.float64) + inp["fa_u_b2"])
    u = 1.0 / (1.0 + np.exp(-(fu @ inp["W_uw"].astype(np.float64)
                              + inp["b_uw"])))                 # [16, 3]

    r_all = inp["memory"][np.arange(B)[:, None], idx]          # [16, 3, 256]
    r_sm = r_all.transpose(1, 0, 2).reshape(P3, SLOT_SIZE)     # step-major
    u_sm = u.T.reshape(-1).astype(np.float32)                  # [48]
    rw_sm = read_w.T.reshape(-1).astype(np.float32)            # [48]

    # ---- launch 2 inputs ----
    cat3 = np.concatenate([r_sm, np.tile(x, (READ_SLOTS, 1))], axis=1)
    cat3T = np.ascontiguousarray(cat3.T)                       # [768, 48]
    ru = (1.0 - u_sm)[:, None] * r_sm                          # [48, 256]
    glue = np.zeros((128, 10, P3), dtype=bf)
    glue[:, 0:6, :] = _pack128(cat3T.astype(bf), 6)
    glue[:, 6:8, :] = _pack128(np.ascontiguousarray(ru.T).astype(bf), 2)
    # natural scale; the 1/WS^2 descale lives in the broadcast lhsT value
    glue[0, 8, :] = u_sm.astype(bf)
    glue[0, 9, :] = rw_sm.astype(bf)

    umw1 = WS * inp["fa_um_w1"]                                # [768, 768]
    xc = (x @ umw1[SLOT_SIZE:] + WS * inp["fa_um_b1"])         # [16, 768]
    xco = np.zeros((16, 816), dtype=bf)
    xco[:, 0:768] = xc.astype(bf)
    xco[:, 768:816] = np.tile(np.eye(B, dtype=bf), (1, READ_SLOTS))

    umw2p = WS * (inp["fa_um_w2"]
                  + np.outer(np.ones(CU, np.float32), inp["fa_um_b2"]))
    amw2p = WS * (inp["fa_am_w2"]
                  + np.outer(np.ones(CA, np.float32), inp["fa_am_b2"]))
    wf8 = np.concatenate([
        _packflat((umw1[:SLOT_SIZE]).astype(f8), 2),
        _packflat(umw2p.astype(f8), 6),
        _packflat((WS * inp["W_um"]).astype(f8), 6),
        _packflat((WS * inp["fa_am_w1"]).astype(f8), 4),
    ], axis=1)
    wbf = np.concatenate([
        _packflat(amw2p.astype(bf), 4),
        _packflat((WS * inp["W_am"]).astype(bf), 4),
    ], axis=1)
    in_map2 = {"glue": glue, "xco": xco, "wf8": wf8, "wbf": wbf}
    if with_bum or with_amb1 or with_bam:
        in_map2["bp2"] = np.concatenate([
            WS * WS * inp["b_um"], WS * inp["fa_am_b1"],
            WS * WS * inp["b_am"],
        ]).reshape(1, -1).astype(np.float32)
    res2 = bass_utils.run_bass_kernel_spmd(nc2, [in_map2], core_ids=[0])
    mT = np.asarray(res2.results[0]["m"], np.float32)          # [128, 2, 16]
    m = mT.transpose(1, 0, 2).reshape(SLOT_SIZE, B).T          # [16, 256]
    return np.tanh(m)
